# revision 1
# baseline (speedup 1.0000x reference)
"""Trainium2 Bass kernel for nn_NestRQModel (NEST-RQ pretraining loss).

Strategy: pure data-parallel over batch (2 batches per core, 8 cores), no
collectives.  Each core computes partial masked sums (nll, corr) and a
vocab-presence bitmap; the host combines them into the 4 scalar outputs.

Per-core pipeline (1024 rows = 2 batches x 512 frames):
  stage 0: LayerNorm stats (bn_stats) on stacked frames [128,320/tile];
           projection matmul q = stack @ P with LN folded in afterwards as a
           per-row affine fix  x = s*(q - mu*colsum(P))  (LN commutes through
           the linear projection).  Row-norm of x is skipped entirely: codes
           are an argmax over cosine-monotone scores, and the e2 term of the
           reference distance varies by < 1e-8 across the codebook.
  stage A: dots = x @ E^T as ONE K=64 matmul per tile via bf16 hi/lo Kahan
           stacking (error ~1e-6, full PE rate); argmax via DVE max/max_index.
  stage B: logits = enc @ W in float32r (full PE rate, N=512);
           ACT Exp with accum_out -> row sum-of-exp (no max subtraction
           needed: |logits| < ~4 so exp cannot overflow);
           corr  = [#(exp(logits) > exp(tgt+margin)) == 0] via one 2x-mode
           tensor_scalar is_gt with accum;
           tgt logit = indirect-DMA gather of W^T rows + fused TT-reduce dot.
  presence: indirect-DMA scatter of 1.0 at (code*mask) indices — identical
           semantics to reference's presence[masked_tgt]=1 (masked rows
           scatter index 0).
"""
import os
import sys

import numpy as np

os.environ.setdefault("MYCRO_LOCAL_CACHE", "1")

try:
    import concourse.bass as bass
except ImportError:
    sys.path.insert(0, "/opt/trn_rl_repo")
    import concourse.bass as bass

import ml_dtypes
import concourse.bacc as bacc
import concourse.tile as tile
from concourse import mybir
from concourse.bass import IndirectOffsetOnAxis
from concourse.masks import make_identity
from contextlib import ExitStack

F32 = mybir.dt.float32
F32R = mybir.dt.float32r
BF16 = mybir.dt.bfloat16
U32 = mybir.dt.uint32
I32 = mybir.dt.int32
AF = mybir.ActivationFunctionType
ALU = mybir.AluOpType

# problem constants
NCORES = 8
B, T, F = 16, 2048, 80
STK, STRIDE = 4, 4
N = 512                 # frames per batch after subsampling
SF = STK * F            # 320 stacked feature dim
EDIM = 16
V = 8192
D = 512                 # encoder dim
BLOC = B // NCORES      # 2 batches per core
R = BLOC * N            # 1024 rows per core
RT = R // 128           # 8 row tiles
VC = 2048               # vocab/psum chunk
NVC = V // VC           # 4
EPS_LN = 1e-6
MARGIN = 5e-3           # corr margin: covers bf16 matmul noise (~1.5e-3)

_NC_CACHE = {}


def _build_program():
    if "nc" in _NC_CACHE:
        return _NC_CACHE["nc"]
    nc = bacc.Bacc("TRN2", target_bir_lowering=False)

    stack_rows = nc.declare_dram_parameter("stack_rows", [R, SF], F32, isOutput=False)
    stackT = nc.declare_dram_parameter("stackT", [BLOC, SF, N], F32, isOutput=False)
    proj = nc.declare_dram_parameter("proj", [SF, EDIM], F32, isOutput=False)
    projsum = nc.declare_dram_parameter("projsum", [1, EDIM], F32, isOutput=False)
    ek = nc.declare_dram_parameter("Ek", [128, V], BF16, isOutput=False)
    w = nc.declare_dram_parameter("W", [D, V], BF16, isOutput=False)
    wt = nc.declare_dram_parameter("Wt", [V, D], F32, isOutput=False)
    encT = nc.declare_dram_parameter("encT", [D, R], BF16, isOutput=False)
    enc_rows = nc.declare_dram_parameter("enc_rows", [R, D], F32, isOutput=False)
    maskce = nc.declare_dram_parameter("maskce", [R, 1], F32, isOutput=False)

    out_stats = nc.declare_dram_parameter("out_stats", [1, 8], F32, isOutput=True)
    out_pres = nc.declare_dram_parameter("out_pres", [V, 1], F32, isOutput=True)

    codes_dram = nc.dram_tensor("codes_scratch", [R, 1], U32)

    with tile.TileContext(nc) as tc, ExitStack() as ctx:
        const_p = ctx.enter_context(tc.tile_pool(name="const", bufs=1))
        small_p = ctx.enter_context(tc.tile_pool(name="small", bufs=4))
        cols_p = ctx.enter_context(tc.tile_pool(name="cols", bufs=1))
        stage0_p = ctx.enter_context(tc.tile_pool(name="stage0", bufs=2))
        dots_p = ctx.enter_context(tc.tile_pool(name="dots", bufs=2))
        scr_p = ctx.enter_context(tc.tile_pool(name="scr", bufs=2))
        psum_p = ctx.enter_context(tc.tile_pool(name="ps", bufs=2, space="PSUM"))

        count_eng = nc.gpsimd if os.environ.get("NESTRQ_COUNT", "dve") == "gpsimd" \
            else nc.vector

        # ---------------- constants / persistent tiles ----------------
        ident = const_p.tile([128, 128], F32)
        make_identity(nc, ident[:])
        eps_t = const_p.tile([128, 1], F32)
        nc.vector.memset(eps_t[:], EPS_LN)
        ones_t = const_p.tile([128, 1], F32)
        nc.vector.memset(ones_t[:], 1.0)
        csum_b = const_p.tile([128, EDIM], F32)
        _ps_ap = projsum[:]
        nc.sync.dma_start(
            csum_b[:],
            bass.AP(tensor=_ps_ap.tensor, offset=_ps_ap.offset,
                    ap=[[0, 128], _ps_ap.ap[-1]]))
        ek_sb = const_p.tile([128, V], BF16)
        nc.sync.dma_start(ek_sb[:], ek[:])
        xk_bufs = []
        for i_ in range(RT):
            xkt = const_p.tile([128, 128], BF16, name=f"xk_{i_}")
            nc.vector.memset(xkt[:], 0.0)
            xk_bufs.append(xkt)

        # W fully resident (bf16): 4 k-chunks [128, V]
        w_sb = []
        for kc in range(4):
            wt_ = const_p.tile([128, V], BF16, name=f"w_sb_{kc}")
            nc.sync.dma_start(wt_[:], w[kc * 128:(kc + 1) * 128, :])
            w_sb.append(wt_)

        # projection chunks [128,16] x2 + [64,16]
        pj0 = const_p.tile([128, EDIM], F32)
        pj1 = const_p.tile([128, EDIM], F32)
        pj2 = const_p.tile([64, EDIM], F32)
        nc.sync.dma_start(pj0[:], proj[0:128, :])
        nc.sync.dma_start(pj1[:], proj[128:256, :])
        nc.sync.dma_start(pj2[:], proj[256:320, :])
        pj = [pj0, pj1, pj2]

        st_sb = []
        for b_ in range(BLOC):
            row = []
            for kc, (k0, k1) in enumerate([(0, 128), (128, 256), (256, 320)]):
                t_ = const_p.tile([k1 - k0, N], F32, name=f"stackT_{b_}_{kc}")
                nc.sync.dma_start(t_[:], stackT[b_, k0:k1, :])
                row.append(t_)
            st_sb.append(row)

        et_sb = []
        for kc in range(4):
            t_ = const_p.tile([128, R], BF16, name=f"encT_{kc}")
            nc.sync.dma_start(t_[:], encT[kc * 128:(kc + 1) * 128, :])
            et_sb.append(t_)

        s_cols = cols_p.tile([128, RT, NVC], F32)
        c_cols = cols_p.tile([128, RT, NVC], F32)
        ltgt_cols = cols_p.tile([128, RT], F32)
        ethr_cols = cols_p.tile([128, RT], F32)
        mask_cols = cols_p.tile([128, RT], F32)
        red_cols = cols_p.tile([128, 2 * RT], F32)

        # ---------------- stage 0: x Kahan tiles for all row tiles ------
        for rt in range(RT):
            b_ = rt // 4
            c0 = (rt % 4) * 128

            stk_t = stage0_p.tile([128, SF], F32, name="stk_t")
            nc.sync.dma_start(stk_t[:], stack_rows[rt * 128:(rt + 1) * 128, :])
            stats = small_p.tile([128, 6], F32, name="stats")
            nc.vector.bn_stats(stats[:], stk_t[:])
            mv = small_p.tile([128, 2], F32, name="mv")
            nc.vector.bn_aggr(mv[:], stats[:])
            rstd = small_p.tile([128, 1], F32, name="rstd")
            nc.scalar.activation(rstd[:], mv[:, 1:2], AF.Sqrt, bias=eps_t[:])
            nc.vector.reciprocal(rstd[:], rstd[:])

            psq = psum_p.tile([128, EDIM], F32, name="psq", tag="big")
            for kc, (k0, k1) in enumerate([(0, 128), (128, 256), (256, 320)]):
                nc.tensor.matmul(psq[:], st_sb[b_][kc][:, c0:c0 + 128], pj[kc][:],
                                 start=(kc == 0), stop=(kc == 2))
            mu_c = small_p.tile([128, EDIM], F32, name="mu_c")
            nc.vector.tensor_scalar(mu_c[:], csum_b[:], mv[:, 0:1], None, ALU.mult)
            x_t = small_p.tile([128, EDIM], F32, name="x_t")
            nc.vector.tensor_tensor(out=x_t[:], in0=psq[:], in1=mu_c[:],
                                    op=ALU.subtract)
            nc.vector.tensor_scalar(x_t[:], x_t[:], rstd[:], None, ALU.mult)

            pst = psum_p.tile([16, 128], F32, name="pst", tag="big")
            nc.tensor.transpose(pst[:], x_t[:], ident[:])

            xk = xk_bufs[rt]
            xh_f = small_p.tile([16, 128], F32, name="xh_f")
            nc.vector.tensor_copy(xk[0:16, :], pst[:])           # hi (cast)
            nc.vector.tensor_copy(xh_f[:], xk[0:16, :])          # back to f32
            nc.vector.tensor_tensor(out=xh_f[:], in0=pst[:], in1=xh_f[:],
                                    op=ALU.subtract)             # residual
            nc.vector.tensor_copy(xk[32:48, :], xh_f[:])         # lo (cast)
            nc.vector.tensor_copy(xk[64:80, :], xk[0:16, :])
            nc.vector.tensor_copy(xk[96:112, :], xk[32:48, :])

        # ---------------- pipelined stage A(rt+1) / prep(rt) / B(rt) ----
        tgt_i = [None] * RT

        def emit_stage_a(rt):
            xk = xk_bufs[rt]
            dots_sb = dots_p.tile([128, V], F32, name="dots_sb")
            for h in range(NVC):
                psd = psum_p.tile([128, VC], F32, name="psd", tag="big")
                for j in range(VC // 512):
                    nc.tensor.matmul(
                        psd[:, j * 512:(j + 1) * 512], xk[:],
                        ek_sb[:, h * VC + j * 512:h * VC + (j + 1) * 512],
                        start=True, stop=True)
                nc.scalar.activation(dots_sb[:, h * VC:(h + 1) * VC], psd[:],
                                     AF.Copy)
            m8 = small_p.tile([128, 8], F32, name="m8")
            i8 = small_p.tile([128, 8], U32, name="i8")
            nc.vector.max(m8[:], dots_sb[:])
            nc.vector.max_index(i8[:], m8[:], dots_sb[:])
            nc.sync.dma_start(codes_dram[rt * 128:(rt + 1) * 128, :], i8[:, 0:1])

        def emit_prep(rt):
            tgt_t = small_p.tile([128, 1], U32, name=f"tgt_{rt}", bufs=RT)
            nc.sync.dma_start(tgt_t[:], codes_dram[rt * 128:(rt + 1) * 128, :])
            tgt_i[rt] = tgt_t
            nc.sync.dma_start(mask_cols[:, rt:rt + 1],
                              maskce[rt * 128:(rt + 1) * 128, :])
            g_t = scr_p.tile([128, D], F32, name="g_t")
            nc.gpsimd.indirect_dma_start(
                out=g_t[:], out_offset=None, in_=wt[:],
                in_offset=IndirectOffsetOnAxis(ap=tgt_t[:, :1], axis=0))
            er_t = scr_p.tile([128, D], F32, name="er_t")
            nc.sync.dma_start(er_t[:], enc_rows[rt * 128:(rt + 1) * 128, :])
            prod = scr_p.tile([128, D], F32, name="prod")
            nc.vector.tensor_tensor(out=prod[:], in0=er_t[:], in1=g_t[:],
                                    op=ALU.mult)
            nc.vector.reduce_sum(ltgt_cols[:, rt:rt + 1], prod[:],
                                 axis=mybir.AxisListType.X)
            thr = small_p.tile([128, 1], F32, name="thr")
            nc.vector.tensor_scalar(thr[:], ltgt_cols[:, rt:rt + 1], MARGIN, None,
                                    ALU.add)
            nc.scalar.activation(ethr_cols[:, rt:rt + 1], thr[:], AF.Exp)

        def emit_stage_b(rt):
            for h in range(NVC):
                psl = psum_p.tile([128, VC], F32, name="psl", tag="big")
                for kc in range(4):
                    for j in range(VC // 512):
                        nc.tensor.matmul(
                            psl[:, j * 512:(j + 1) * 512],
                            et_sb[kc][:, rt * 128:(rt + 1) * 128],
                            w_sb[kc][:, h * VC + j * 512:h * VC + (j + 1) * 512],
                            start=(kc == 0), stop=(kc == 3))
                exp_t = scr_p.tile([128, VC], BF16, name="exp_t")
                nc.scalar.activation(exp_t[:], psl[:], AF.Exp,
                                     accum_out=s_cols[:, rt, h:h + 1])
                gt_t = scr_p.tile([128, VC], BF16, name="gt_t")
                count_eng.tensor_scalar(gt_t[:], exp_t[:],
                                        ethr_cols[:, rt:rt + 1], None,
                                        ALU.is_gt, ALU.add,
                                        accum_out=c_cols[:, rt, h:h + 1])

        emit_stage_a(0)
        emit_stage_a(1)
        emit_prep(0)
        for rt in range(RT):
            emit_stage_b(rt)
            if rt + 2 < RT:
                emit_stage_a(rt + 2)
            if rt + 1 < RT:
                emit_prep(rt + 1)

        # ---------------- finalize per row tile -------------------------
        for rt in range(RT):
            s_t = small_p.tile([128, 1], F32, name="s_t")
            nc.vector.reduce_sum(s_t[:], s_cols[:, rt, :], axis=mybir.AxisListType.X)
            cnt_t = small_p.tile([128, 1], F32, name="cnt_t")
            nc.vector.reduce_sum(cnt_t[:], c_cols[:, rt, :], axis=mybir.AxisListType.X)
            lnS = small_p.tile([128, 1], F32, name="lnS")
            nc.scalar.activation(lnS[:], s_t[:], AF.Ln)
            nll = small_p.tile([128, 1], F32, name="nll")
            nc.vector.tensor_tensor(out=nll[:], in0=lnS[:],
                                    in1=ltgt_cols[:, rt:rt + 1], op=ALU.subtract)
            nc.vector.tensor_tensor(out=red_cols[:, rt:rt + 1], in0=nll[:],
                                    in1=mask_cols[:, rt:rt + 1], op=ALU.mult)
            corr = small_p.tile([128, 1], F32, name="corr")
            nc.vector.tensor_scalar(corr[:], cnt_t[:], 0.5, None, ALU.is_lt)
            nc.vector.tensor_tensor(out=red_cols[:, RT + rt:RT + rt + 1],
                                    in0=corr[:], in1=mask_cols[:, rt:rt + 1],
                                    op=ALU.mult)

            pidx_f = small_p.tile([128, 1], F32, name="pidx_f")
            nc.vector.tensor_copy(pidx_f[:], tgt_i[rt][:, :1])
            nc.vector.tensor_tensor(out=pidx_f[:], in0=pidx_f[:],
                                    in1=mask_cols[:, rt:rt + 1], op=ALU.mult)
            pidx = small_p.tile([128, 1], I32, name="pidx")
            nc.vector.tensor_copy(pidx[:], pidx_f[:])
            p0 = 1 if rt % 4 == 0 else 0
            nc.gpsimd.indirect_dma_start(
                out=out_pres[:], out_offset=IndirectOffsetOnAxis(
                    ap=pidx[p0:128, :1], axis=0),
                in_=ones_t[p0:128, :], in_offset=None)

        # ---------------- partition reduction ---------------------------
        psr = psum_p.tile([1, 2 * RT], F32, name="psr", tag="big")
        nc.tensor.matmul(psr[:], ones_t[:], red_cols[:], start=True, stop=True)
        fin = small_p.tile([1, 8], F32, name="fin")
        nc.vector.reduce_sum(fin[:, 0:1], psr[0:1, 0:RT], axis=mybir.AxisListType.X)
        nc.vector.reduce_sum(fin[:, 1:2], psr[0:1, RT:2 * RT],
                             axis=mybir.AxisListType.X)
        nc.vector.memset(fin[:, 2:8], 0.0)
        nc.sync.dma_start(out_stats[:], fin[:])

    nc.compile()
    _NC_CACHE["nc"] = nc
    return nc


def _prep_core_inputs(inputs, core):
    feats = inputs["feats"]
    lengths = inputs["feats_lengths"]
    enc = inputs["encoder_out"]
    proj = inputs["projection"]
    emb = inputs["embeddings"]
    top = inputs["top_n_out"]

    b0 = core * BLOC
    fb = np.ascontiguousarray(feats[b0:b0 + BLOC]).reshape(BLOC, N, SF)
    stack_rows = np.ascontiguousarray(fb.reshape(R, SF), dtype=np.float32)
    stackT = np.ascontiguousarray(fb.transpose(0, 2, 1), dtype=np.float32)

    encb = enc[b0:b0 + BLOC].reshape(R, D).astype(np.float32)
    # row r of the CE grid uses enc frame t = r-1 (shifted so that row r's
    # target is codes[r], removing the cross-tile dependency)
    enc_shift = np.zeros((R, D), np.float32)
    enc_shift[1:] = encb[:-1]
    encT = np.ascontiguousarray(enc_shift.T.astype(ml_dtypes.bfloat16))

    L = (lengths[b0:b0 + BLOC].astype(np.int64) // STRIDE)
    r_idx = np.arange(R)
    tval = r_idx - 1
    tloc = tval % N
    tb = np.clip(tval // N, 0, BLOC - 1)
    maskce = ((tval >= 0) & (tloc != N - 1)
              & (tloc < L[tb] - 1)).astype(np.float32).reshape(R, 1)

    return {
        "stack_rows": stack_rows,
        "stackT": stackT,
        "encT": encT,
        "enc_rows": np.ascontiguousarray(enc_shift),
        "maskce": np.ascontiguousarray(maskce),
    }


def _prep_shared_inputs(inputs):
    proj = np.asarray(inputs["projection"], dtype=np.float32)
    emb = np.asarray(inputs["embeddings"], dtype=np.float32)
    top = np.asarray(inputs["top_n_out"], dtype=np.float32)

    projsum = proj.sum(0, keepdims=True).astype(np.float32)  # [1, 16]

    Et = np.ascontiguousarray(emb[:, 0, :].T, dtype=np.float32)  # [16, V]
    Eh = Et.astype(ml_dtypes.bfloat16).astype(np.float32)
    El = (Et - Eh).astype(ml_dtypes.bfloat16).astype(np.float32)
    Z = np.zeros_like(Eh)
    # row pairing with x tile [xh;0;xl;0;xh;0;xl;0]: hh + lh + hl + ll
    Ek = np.concatenate(
        [Eh, Z, Eh, Z, El, Z, El, Z], axis=0).astype(ml_dtypes.bfloat16)

    W = np.ascontiguousarray(top[0, 0], dtype=np.float32)        # [D, V]
    Wt = np.ascontiguousarray(W.T)                               # [V, D]
    return {
        "proj": np.ascontiguousarray(proj),
        "projsum": projsum,
        "Ek": np.ascontiguousarray(Ek),
        "W": np.ascontiguousarray(W.astype(ml_dtypes.bfloat16)),
        "Wt": Wt,
    }


def _combine(results, inputs):
    lengths = np.asarray(inputs["feats_lengths"]).astype(np.int64)
    L = lengths // STRIDE
    num_codes = float((L - 1).sum())

    nll_sum = 0.0
    corr_sum = 0.0
    pres = np.zeros(V, dtype=bool)
    for r in results:
        st = np.asarray(r["out_stats"]).reshape(-1)
        nll_sum += float(st[0])
        corr_sum += float(st[1])
        pres |= np.asarray(r["out_pres"]).reshape(-1) > 0.0

    loss = np.float32(nll_sum / num_codes)
    acc = np.float32(corr_sum / num_codes)
    uniq = np.float32(pres.sum())
    return np.array([loss, acc, np.float32(num_codes), uniq], dtype=np.float32)


def _run(inputs, trace=False):
    from concourse.bass_utils import run_bass_kernel_spmd
    nc = _build_program()
    shared = _prep_shared_inputs(inputs)
    in_maps = []
    for core in range(NCORES):
        m = dict(shared)
        m.update(_prep_core_inputs(inputs, core))
        in_maps.append(m)
    res = run_bass_kernel_spmd(nc, in_maps, core_ids=list(range(NCORES)),
                               trace=trace)
    out = _combine(res.results, inputs)
    return out, res


def _run_sim(inputs, core=0):
    """Single-core simulator run (correctness debugging)."""
    from concourse.bass_interp import CoreSim
    nc = _build_program()
    m = dict(_prep_shared_inputs(inputs))
    m.update(_prep_core_inputs(inputs, core))
    sim = CoreSim(nc)
    for k, v in m.items():
        sim.tensor(k)[:] = v
    sim.simulate()
    return {k: np.array(sim.tensor(k)) for k in ("out_stats", "out_pres")}


def kernel(**inputs) -> np.ndarray:
    out, _ = _run(inputs, trace=False)
    return out



# revision 3
# speedup vs baseline: 1.2282x; 1.2282x over previous
"""Trainium2 Bass kernel for nn_NestRQModel (NEST-RQ pretraining loss).

Strategy: data-parallel over COMPACTED valid CE rows.  The reference only
counts rows (b, j) with j+1 < lengths[b]//4 (5700 of 8176); the host builds
that row list from feats_lengths (pure reindexing) and pads to 6144 = 8
cores x 768 rows.  Each core computes partial masked sums (nll, corr) and a
vocab-presence scatter; the host combines into the 4 scalar outputs.

Per-core pipeline (768 rows = 6 row tiles):
  stage 0: LayerNorm stats (bn_stats); projection matmul with LN folded in
           afterwards as a per-row affine fix (LN commutes through the
           linear map); rstd = exp(-0.5*ln(var+eps)) on ACT (keeps the
           natural_log_exp table set resident -- no Sqrt table thrash);
           x packed to bf16 hi/lo Kahan rows for stage A.
  stage A: dots = x @ E^T as K=128 Kahan bf16 matmuls (error ~1e-6, full
           PE rate); argmax runs DIRECTLY ON PSUM: per 2048-chunk
           max8/find_index8, then a cheap cross-chunk combine (the 8.6us
           ACT copy of dots to SBUF is gone entirely).
  stage B: logits = enc @ W bf16 (full PE rate); ACT Exp with accum_out
           -> row sum-of-exp; corr via a SECOND sharp exp on ACT:
           sum_v exp(S*(logit_v - thr)) < 0.5  <=>  no logit above
           thr = ltgt + margin  (overflow -> inf -> counted incorrect,
           which is the right answer);  tgt logit = indirect-DMA gather
           of W^T rows + fused dot.
  presence: indirect-DMA scatter of 1.0 at (code*mask) indices.
"""
import os
import sys

import numpy as np

os.environ.setdefault("MYCRO_LOCAL_CACHE", "1")

try:
    import concourse.bass as bass
except ImportError:
    sys.path.insert(0, "/opt/trn_rl_repo")
    import concourse.bass as bass

import ml_dtypes
import concourse.bacc as bacc
import concourse.tile as tile
from concourse import mybir
from concourse.bass import IndirectOffsetOnAxis
from concourse.masks import make_identity
from contextlib import ExitStack

F32 = mybir.dt.float32
BF16 = mybir.dt.bfloat16
U32 = mybir.dt.uint32
I32 = mybir.dt.int32
AF = mybir.ActivationFunctionType
ALU = mybir.AluOpType

# problem constants
NCORES = 8
B, T, F = 16, 2048, 80
STK, STRIDE = 4, 4
N = 512                 # frames per batch after subsampling
SF = STK * F            # 320 stacked feature dim
EDIM = 16
V = 8192
D = 512                 # encoder dim
R = 768                 # compacted rows per core (8*768 = 6144 >= 5700)
RT = R // 128           # 6 row tiles
VC = 2048               # vocab/psum chunk
NVC = V // VC           # 4
EPS_LN = 1e-6
MARGIN = 7e-3           # corr margin: covers bf16 matmul noise (~1.5e-3)
SHARP = 400.0           # corr sharp-exp scale

_NC_CACHE = {}


def _build_program():
    if "nc" in _NC_CACHE:
        return _NC_CACHE["nc"]
    nc = bacc.Bacc("TRN2", target_bir_lowering=False)

    stack_rows = nc.declare_dram_parameter("stack_rows", [R, SF], F32, isOutput=False)
    stackT = nc.declare_dram_parameter("stackT", [SF, R], F32, isOutput=False)
    proj = nc.declare_dram_parameter("proj", [SF, EDIM], F32, isOutput=False)
    projsum = nc.declare_dram_parameter("projsum", [1, EDIM], F32, isOutput=False)
    ek = nc.declare_dram_parameter("Ek", [128, V], BF16, isOutput=False)
    w = nc.declare_dram_parameter("W", [D, V], BF16, isOutput=False)
    wt = nc.declare_dram_parameter("Wt", [V, D], F32, isOutput=False)
    encT = nc.declare_dram_parameter("encT", [D, R], BF16, isOutput=False)
    enc_rows = nc.declare_dram_parameter("enc_rows", [R, D], F32, isOutput=False)
    maskce = nc.declare_dram_parameter("maskce", [R, 1], F32, isOutput=False)

    out_stats = nc.declare_dram_parameter("out_stats", [1, 8], F32, isOutput=True)
    out_pres = nc.declare_dram_parameter("out_pres", [V, 1], F32, isOutput=True)

    with tile.TileContext(nc) as tc, ExitStack() as ctx:
        const_p = ctx.enter_context(tc.tile_pool(name="const", bufs=1))
        small_p = ctx.enter_context(tc.tile_pool(name="small", bufs=4))
        cols_p = ctx.enter_context(tc.tile_pool(name="cols", bufs=1))
        stage0_p = ctx.enter_context(tc.tile_pool(name="stage0", bufs=2))
        scans_p = ctx.enter_context(tc.tile_pool(name="scans", bufs=2))
        scr_p = ctx.enter_context(tc.tile_pool(name="scr", bufs=2))
        psum_p = ctx.enter_context(tc.tile_pool(name="ps", bufs=2, space="PSUM"))

        # ---------------- constants / persistent tiles ----------------
        ident = const_p.tile([128, 128], F32)
        make_identity(nc, ident[:])
        eps_t = const_p.tile([128, 1], F32)
        nc.vector.memset(eps_t[:], EPS_LN)
        ones_t = const_p.tile([128, 1], F32)
        nc.vector.memset(ones_t[:], 1.0)
        csum_b = const_p.tile([128, EDIM], F32)
        _ps_ap = projsum[:]
        nc.sync.dma_start(
            csum_b[:],
            bass.AP(tensor=_ps_ap.tensor, offset=_ps_ap.offset,
                    ap=[[0, 128], _ps_ap.ap[-1]]))
        ek_sb = const_p.tile([128, V], BF16)
        nc.sync.dma_start(ek_sb[:], ek[:])
        xk_bufs = []
        for i_ in range(RT):
            xkt = const_p.tile([128, 128], BF16, name=f"xk_{i_}")
            nc.vector.memset(xkt[:], 0.0)
            xk_bufs.append(xkt)

        # W fully resident (bf16): 4 k-chunks [128, V]
        w_sb = []
        for kc in range(4):
            wt_ = const_p.tile([128, V], BF16, name=f"w_sb_{kc}")
            nc.sync.dma_start(wt_[:], w[kc * 128:(kc + 1) * 128, :])
            w_sb.append(wt_)

        # projection chunks [128,16] x2 + [64,16]
        pj0 = const_p.tile([128, EDIM], F32)
        pj1 = const_p.tile([128, EDIM], F32)
        pj2 = const_p.tile([64, EDIM], F32)
        nc.sync.dma_start(pj0[:], proj[0:128, :])
        nc.sync.dma_start(pj1[:], proj[128:256, :])
        nc.sync.dma_start(pj2[:], proj[256:320, :])
        pj = [pj0, pj1, pj2]

        st_sb = []
        for kc, (k0, k1) in enumerate([(0, 128), (128, 256), (256, 320)]):
            t_ = const_p.tile([k1 - k0, R], F32, name=f"stackT_{kc}")
            nc.sync.dma_start(t_[:], stackT[k0:k1, :])
            st_sb.append(t_)

        et_sb = []
        for kc in range(4):
            t_ = const_p.tile([128, R], BF16, name=f"encT_{kc}")
            nc.sync.dma_start(t_[:], encT[kc * 128:(kc + 1) * 128, :])
            et_sb.append(t_)

        s_cols = cols_p.tile([128, RT, NVC], F32)
        c_cols = cols_p.tile([128, RT, NVC], F32)
        ltgt_cols = cols_p.tile([128, RT], F32)
        bias_cols = cols_p.tile([128, RT], F32)
        mask_cols = cols_p.tile([128, RT], F32)
        red_cols = cols_p.tile([128, 2 * RT], F32)

        # ---------------- stage 0: x Kahan tiles for all row tiles ------
        for rt in range(RT):
            c0 = rt * 128

            stk_t = stage0_p.tile([128, SF], F32, name="stk_t")
            nc.sync.dma_start(stk_t[:], stack_rows[rt * 128:(rt + 1) * 128, :])
            stats = small_p.tile([128, 6], F32, name="stats")
            nc.vector.bn_stats(stats[:], stk_t[:])
            mv = small_p.tile([128, 2], F32, name="mv")
            nc.vector.bn_aggr(mv[:], stats[:])
            # rstd = exp(-0.5*ln(var+eps)): stays in the natural_log_exp set
            lnv = small_p.tile([128, 1], F32, name="lnv")
            nc.scalar.activation(lnv[:], mv[:, 1:2], AF.Ln, bias=eps_t[:])
            rstd = small_p.tile([128, 1], F32, name="rstd")
            nc.scalar.activation(rstd[:], lnv[:], AF.Exp, scale=-0.5)

            psq = psum_p.tile([128, EDIM], F32, name="psq", tag="big")
            for kc, (k0, k1) in enumerate([(0, 128), (128, 256), (256, 320)]):
                nc.tensor.matmul(psq[:], st_sb[kc][:, c0:c0 + 128], pj[kc][:],
                                 start=(kc == 0), stop=(kc == 2))
            mu_c = small_p.tile([128, EDIM], F32, name="mu_c")
            nc.vector.tensor_scalar(mu_c[:], csum_b[:], mv[:, 0:1], None, ALU.mult)
            x_t = small_p.tile([128, EDIM], F32, name="x_t")
            nc.vector.tensor_tensor(out=x_t[:], in0=psq[:], in1=mu_c[:],
                                    op=ALU.subtract)
            nc.vector.tensor_scalar(x_t[:], x_t[:], rstd[:], None, ALU.mult)

            pst = psum_p.tile([16, 128], F32, name="pst", tag="big")
            nc.tensor.transpose(pst[:], x_t[:], ident[:])

            xk = xk_bufs[rt]
            xh_f = small_p.tile([16, 128], F32, name="xh_f")
            nc.vector.tensor_copy(xk[0:16, :], pst[:])           # hi (cast)
            nc.vector.tensor_copy(xh_f[:], xk[0:16, :])          # back to f32
            nc.vector.tensor_tensor(out=xh_f[:], in0=pst[:], in1=xh_f[:],
                                    op=ALU.subtract)             # residual
            nc.vector.tensor_copy(xk[32:48, :], xh_f[:])         # lo (cast)
            nc.vector.tensor_copy(xk[64:80, :], xk[0:16, :])
            nc.vector.tensor_copy(xk[96:112, :], xk[32:48, :])

        # ---------------- pipelined stage A(rt+1) / prep(rt) / B(rt) ----
        codes_i = [None] * RT
        codes_f = [None] * RT

        def emit_stage_a(rt):
            xk = xk_bufs[rt]
            mcat = scans_p.tile([128, 32], F32, name="mcat")
            icat = scans_p.tile([128, 32], U32, name="icat")
            for h in range(NVC):
                psd = psum_p.tile([128, VC], F32, name="psd", tag="big")
                for j in range(VC // 512):
                    nc.tensor.matmul(
                        psd[:, j * 512:(j + 1) * 512], xk[:],
                        ek_sb[:, h * VC + j * 512:h * VC + (j + 1) * 512],
                        start=True, stop=True)
                nc.vector.max(mcat[:, 8 * h:8 * h + 8], psd[:])
                nc.vector.max_index(icat[:, 8 * h:8 * h + 8],
                                    mcat[:, 8 * h:8 * h + 8], psd[:])
            # cross-chunk combine: global max position p = 8*h_win in mcat
            gm8 = small_p.tile([128, 8], F32, name="gm8")
            nc.vector.max(gm8[:], mcat[:])
            gp8 = small_p.tile([128, 8], U32, name="gp8")
            nc.vector.max_index(gp8[:], gm8[:], mcat[:])
            pf = small_p.tile([128, 1], F32, name="pf")
            nc.vector.tensor_copy(pf[:], gp8[:, 0:1])
            cf = small_p.tile([128, 1], F32, name=f"cf_{rt}", bufs=RT)
            nc.vector.memset(cf[:], 0.0)
            eq = small_p.tile([128, 1], F32, name="eq")
            lidx = small_p.tile([128, 1], F32, name="lidx")
            term = small_p.tile([128, 1], F32, name="term")
            for h in range(NVC):
                nc.vector.tensor_scalar(eq[:], pf[:], float(8 * h), None,
                                        ALU.is_equal)
                nc.vector.tensor_copy(lidx[:], icat[:, 8 * h:8 * h + 1])
                nc.vector.scalar_tensor_tensor(
                    out=term[:], in0=lidx[:], scalar=float(VC * h),
                    in1=eq[:], op0=ALU.add, op1=ALU.mult)
                nc.vector.tensor_tensor(out=cf[:], in0=cf[:], in1=term[:],
                                        op=ALU.add)
            ci = small_p.tile([128, 1], U32, name=f"ci_{rt}", bufs=RT)
            nc.vector.tensor_copy(ci[:], cf[:])
            codes_i[rt] = ci
            codes_f[rt] = cf

        def emit_prep(rt):
            nc.sync.dma_start(mask_cols[:, rt:rt + 1],
                              maskce[rt * 128:(rt + 1) * 128, :])
            g_t = scr_p.tile([128, D], F32, name="g_t")
            nc.gpsimd.indirect_dma_start(
                out=g_t[:], out_offset=None, in_=wt[:],
                in_offset=IndirectOffsetOnAxis(ap=codes_i[rt][:, :1], axis=0))
            er_t = scr_p.tile([128, D], F32, name="er_t")
            nc.sync.dma_start(er_t[:], enc_rows[rt * 128:(rt + 1) * 128, :])
            prod = scr_p.tile([128, D], F32, name="prod")
            nc.vector.tensor_tensor(out=prod[:], in0=er_t[:], in1=g_t[:],
                                    op=ALU.mult)
            nc.vector.reduce_sum(ltgt_cols[:, rt:rt + 1], prod[:],
                                 axis=mybir.AxisListType.X)
            # exp2 bias = -SHARP*(ltgt + MARGIN)
            nc.vector.tensor_scalar(bias_cols[:, rt:rt + 1],
                                    ltgt_cols[:, rt:rt + 1],
                                    -SHARP, -SHARP * MARGIN,
                                    ALU.mult, ALU.add)

        def emit_stage_b(rt):
            for h in range(NVC):
                psl = psum_p.tile([128, VC], F32, name="psl", tag="big")
                for kc in range(4):
                    for j in range(VC // 512):
                        nc.tensor.matmul(
                            psl[:, j * 512:(j + 1) * 512],
                            et_sb[kc][:, rt * 128:(rt + 1) * 128],
                            w_sb[kc][:, h * VC + j * 512:h * VC + (j + 1) * 512],
                            start=(kc == 0), stop=(kc == 3))
                exp_t = scr_p.tile([128, VC], BF16, name="exp_t")
                nc.scalar.activation(exp_t[:], psl[:], AF.Exp,
                                     accum_out=s_cols[:, rt, h:h + 1])
                shp_t = scr_p.tile([128, VC], BF16, name="shp_t")
                nc.scalar.activation(shp_t[:], psl[:], AF.Exp,
                                     scale=SHARP,
                                     bias=bias_cols[:, rt:rt + 1],
                                     accum_out=c_cols[:, rt, h:h + 1])

        emit_stage_a(0)
        emit_stage_a(1)
        emit_prep(0)
        for rt in range(RT):
            emit_stage_b(rt)
            if rt + 2 < RT:
                emit_stage_a(rt + 2)
            if rt + 1 < RT:
                emit_prep(rt + 1)

        # ---------------- finalize per row tile -------------------------
        for rt in range(RT):
            s_t = small_p.tile([128, 1], F32, name="s_t")
            nc.vector.reduce_sum(s_t[:], s_cols[:, rt, :], axis=mybir.AxisListType.X)
            cnt_t = small_p.tile([128, 1], F32, name="cnt_t")
            nc.vector.reduce_sum(cnt_t[:], c_cols[:, rt, :], axis=mybir.AxisListType.X)
            lnS = small_p.tile([128, 1], F32, name="lnS")
            nc.scalar.activation(lnS[:], s_t[:], AF.Ln)
            nll = small_p.tile([128, 1], F32, name="nll")
            nc.vector.tensor_tensor(out=nll[:], in0=lnS[:],
                                    in1=ltgt_cols[:, rt:rt + 1], op=ALU.subtract)
            nc.vector.tensor_tensor(out=red_cols[:, rt:rt + 1], in0=nll[:],
                                    in1=mask_cols[:, rt:rt + 1], op=ALU.mult)
            corr = small_p.tile([128, 1], F32, name="corr")
            nc.vector.tensor_scalar(corr[:], cnt_t[:], 0.5, None, ALU.is_lt)
            nc.vector.tensor_tensor(out=red_cols[:, RT + rt:RT + rt + 1],
                                    in0=corr[:], in1=mask_cols[:, rt:rt + 1],
                                    op=ALU.mult)

            pidx_f = small_p.tile([128, 1], F32, name="pidx_f")
            nc.vector.tensor_tensor(out=pidx_f[:], in0=codes_f[rt][:],
                                    in1=mask_cols[:, rt:rt + 1], op=ALU.mult)
            pidx = small_p.tile([128, 1], I32, name="pidx")
            nc.vector.tensor_copy(pidx[:], pidx_f[:])
            nc.gpsimd.indirect_dma_start(
                out=out_pres[:], out_offset=IndirectOffsetOnAxis(
                    ap=pidx[:, :1], axis=0),
                in_=ones_t[:, :], in_offset=None)

        # ---------------- partition reduction ---------------------------
        psr = psum_p.tile([1, 2 * RT], F32, name="psr", tag="big")
        nc.tensor.matmul(psr[:], ones_t[:], red_cols[:], start=True, stop=True)
        fin = small_p.tile([1, 8], F32, name="fin")
        nc.vector.reduce_sum(fin[:, 0:1], psr[0:1, 0:RT], axis=mybir.AxisListType.X)
        nc.vector.reduce_sum(fin[:, 1:2], psr[0:1, RT:2 * RT],
                             axis=mybir.AxisListType.X)
        nc.vector.memset(fin[:, 2:8], 0.0)
        nc.sync.dma_start(out_stats[:], fin[:])

    nc.compile()
    _NC_CACHE["nc"] = nc
    return nc


def _row_map(lengths):
    """Valid CE rows (b, j): enc frame j, target frame j+1; j+1 <= L_b-1."""
    L = np.asarray(lengths).astype(np.int64) // STRIDE
    bs, js = [], []
    for b in range(B):
        n = int(L[b]) - 1
        bs.extend([b] * n)
        js.extend(range(n))
    nvalid = len(bs)
    pad = NCORES * R - nvalid
    assert pad >= 0, f"too many valid rows: {nvalid}"
    bs = np.array(bs + [0] * pad, dtype=np.int64)
    js = np.array(js + [0] * pad, dtype=np.int64)
    vm = np.zeros(NCORES * R, dtype=bool)
    vm[:nvalid] = True
    return bs, js, vm, nvalid


def _prep_core_inputs(inputs, core, row_map):
    feats = np.asarray(inputs["feats"])
    enc = np.asarray(inputs["encoder_out"])
    bs, js, vm, _ = row_map
    sl = slice(core * R, (core + 1) * R)
    b_c, j_c, v_c = bs[sl], js[sl], vm[sl]

    fb = feats.reshape(B, N, SF)
    stack_rows = fb[b_c, j_c + 1].astype(np.float32)
    stack_rows[~v_c] = 0.0
    enc_r = enc[b_c, j_c].astype(np.float32)
    enc_r[~v_c] = 0.0
    maskce = v_c.astype(np.float32).reshape(R, 1)

    return {
        "stack_rows": np.ascontiguousarray(stack_rows),
        "stackT": np.ascontiguousarray(stack_rows.T),
        "encT": np.ascontiguousarray(enc_r.T.astype(ml_dtypes.bfloat16)),
        "enc_rows": np.ascontiguousarray(enc_r),
        "maskce": np.ascontiguousarray(maskce),
    }


def _prep_shared_inputs(inputs):
    proj = np.asarray(inputs["projection"], dtype=np.float32)
    emb = np.asarray(inputs["embeddings"], dtype=np.float32)
    top = np.asarray(inputs["top_n_out"], dtype=np.float32)

    projsum = proj.sum(0, keepdims=True).astype(np.float32)  # [1, 16]

    Et = np.ascontiguousarray(emb[:, 0, :].T, dtype=np.float32)  # [16, V]
    Eh = Et.astype(ml_dtypes.bfloat16).astype(np.float32)
    El = (Et - Eh).astype(ml_dtypes.bfloat16).astype(np.float32)
    Z = np.zeros_like(Eh)
    # row pairing with x tile [xh;0;xl;0;xh;0;xl;0]: hh + lh + hl + ll
    Ek = np.concatenate(
        [Eh, Z, Eh, Z, El, Z, El, Z], axis=0).astype(ml_dtypes.bfloat16)

    W = np.ascontiguousarray(top[0, 0], dtype=np.float32)        # [D, V]
    Wt = np.ascontiguousarray(W.T)                               # [V, D]
    return {
        "proj": np.ascontiguousarray(proj),
        "projsum": projsum,
        "Ek": np.ascontiguousarray(Ek),
        "W": np.ascontiguousarray(W.astype(ml_dtypes.bfloat16)),
        "Wt": Wt,
    }


def _combine(results, inputs, row_map):
    _, _, _, nvalid = row_map
    num_codes = float(nvalid)

    nll_sum = 0.0
    corr_sum = 0.0
    pres = np.zeros(V, dtype=bool)
    for r in results:
        st = np.asarray(r["out_stats"]).reshape(-1)
        nll_sum += float(st[0])
        corr_sum += float(st[1])
        pres |= np.asarray(r["out_pres"]).reshape(-1) > 0.0
    # reference scatters index 0 for every masked grid row; those exist
    # whenever num_codes < B*(N-1) (always here)
    if nvalid < B * (N - 1):
        pres[0] = True

    loss = np.float32(nll_sum / num_codes)
    acc = np.float32(corr_sum / num_codes)
    uniq = np.float32(pres.sum())
    return np.array([loss, acc, np.float32(num_codes), uniq], dtype=np.float32)


def _run(inputs, trace=False):
    from concourse.bass_utils import run_bass_kernel_spmd
    nc = _build_program()
    row_map = _row_map(inputs["feats_lengths"])
    shared = _prep_shared_inputs(inputs)
    in_maps = []
    for core in range(NCORES):
        m = dict(shared)
        m.update(_prep_core_inputs(inputs, core, row_map))
        in_maps.append(m)
    res = run_bass_kernel_spmd(nc, in_maps, core_ids=list(range(NCORES)),
                               trace=trace)
    out = _combine(res.results, inputs, row_map)
    return out, res


def _run_sim(inputs, core=0):
    """Single-core simulator run (correctness debugging)."""
    from concourse.bass_interp import CoreSim
    nc = _build_program()
    row_map = _row_map(inputs["feats_lengths"])
    m = dict(_prep_shared_inputs(inputs))
    m.update(_prep_core_inputs(inputs, core, row_map))
    sim = CoreSim(nc, require_finite=False, require_nnan=False)
    for k, v in m.items():
        sim.tensor(k)[:] = v
    sim.simulate()
    return {k: np.array(sim.tensor(k)) for k in ("out_stats", "out_pres")}


def kernel(**inputs) -> np.ndarray:
    out, _ = _run(inputs, trace=False)
    return out


# revision 4
# speedup vs baseline: 1.5152x; 1.2337x over previous
"""Trainium2 Bass kernel for nn_NestRQModel (NEST-RQ pretraining loss).

Strategy: data-parallel over COMPACTED valid CE rows.  The reference only
counts rows (b, j) with j+1 < lengths[b]//4 (5700 of 8176); the host builds
that row list from feats_lengths (pure reindexing) and pads to 6144 = 8
cores x 768 rows.  Each core computes partial masked sums (nll, corr) and a
vocab-presence scatter; the host combines into the 4 scalar outputs.

Per-core pipeline (768 rows = 6 row tiles):
  stage 0: LayerNorm stats (bn_stats); projection matmul with LN folded in
           afterwards as a per-row affine fix (LN commutes through the
           linear map); rstd = exp(-0.5*ln(var+eps)) on ACT, with all Ln's
           batched before all Exp's (one table switch, no Sqrt set);
           x packed to bf16 hi/lo Kahan rows for stage A.
  stage A: dots = x @ E^T as K=128 Kahan bf16 matmuls (error ~1e-6, full
           PE rate); argmax runs DIRECTLY ON PSUM: per 1024-chunk
           max8/find_index8, then a vectorized cross-chunk combine.
  stage B: logits = enc @ W bf16 (full PE rate); ACT Exp with accum_out
           -> row sum-of-exp; corr via a SECOND sharp exp on ACT:
           sum_v exp(S*(logit_v - thr)) < 0.5  <=>  no logit above
           thr = ltgt + margin  (overflow -> inf -> counted incorrect,
           which is the right answer);  tgt logit = indirect-DMA gather
           of W^T rows + fused dot (scalar_tensor_tensor accum).
  PSUM is split: stage-A chunks and stage-B chunks each get their own
  2-buffer [128,1024] pool so DVE scans never block PE/ACT progress.
  presence: indirect-DMA scatter of 1.0 at (code*mask) indices.
"""
import os
import sys

import numpy as np

os.environ.setdefault("MYCRO_LOCAL_CACHE", "1")

try:
    import concourse.bass as bass
except ImportError:
    sys.path.insert(0, "/opt/trn_rl_repo")
    import concourse.bass as bass

import ml_dtypes
import concourse.bacc as bacc
import concourse.tile as tile
from concourse import mybir
from concourse.bass import IndirectOffsetOnAxis
from concourse.masks import make_identity
from contextlib import ExitStack

F32 = mybir.dt.float32
BF16 = mybir.dt.bfloat16
U32 = mybir.dt.uint32
I32 = mybir.dt.int32
AF = mybir.ActivationFunctionType
ALU = mybir.AluOpType

# problem constants
NCORES = 8
B, T, F = 16, 2048, 80
STK, STRIDE = 4, 4
N = 512                 # frames per batch after subsampling
SF = STK * F            # 320 stacked feature dim
EDIM = 16
V = 8192
D = 512                 # encoder dim
R = 768                 # compacted rows per core (8*768 = 6144 >= 5700)
RT = R // 128           # 6 row tiles
VC = 1024               # vocab/psum chunk (2 PSUM banks)
NVC = V // VC           # 8
EPS_LN = 1e-6
MARGIN = 7e-3           # corr margin: covers bf16 matmul noise (~1.5e-3)
SHARP = 400.0           # corr sharp-exp scale

_NC_CACHE = {}


def _build_program():
    if "nc" in _NC_CACHE:
        return _NC_CACHE["nc"]
    nc = bacc.Bacc("TRN2", target_bir_lowering=False)

    stack_rows = nc.declare_dram_parameter("stack_rows", [R, SF], F32, isOutput=False)
    stackT = nc.declare_dram_parameter("stackT", [SF, R], F32, isOutput=False)
    proj = nc.declare_dram_parameter("proj", [SF, EDIM], F32, isOutput=False)
    projsum = nc.declare_dram_parameter("projsum", [1, EDIM], F32, isOutput=False)
    ek = nc.declare_dram_parameter("Ek", [128, V], BF16, isOutput=False)
    w = nc.declare_dram_parameter("W", [D, V], BF16, isOutput=False)
    wt = nc.declare_dram_parameter("Wt", [V, D], F32, isOutput=False)
    encT = nc.declare_dram_parameter("encT", [D, R], BF16, isOutput=False)
    enc_rows = nc.declare_dram_parameter("enc_rows", [R, D], F32, isOutput=False)
    maskce = nc.declare_dram_parameter("maskce", [R, 1], F32, isOutput=False)

    out_stats = nc.declare_dram_parameter("out_stats", [1, 8], F32, isOutput=True)
    out_pres = nc.declare_dram_parameter("out_pres", [V, 1], F32, isOutput=True)

    with tile.TileContext(nc) as tc, ExitStack() as ctx:
        const_p = ctx.enter_context(tc.tile_pool(name="const", bufs=1))
        small_p = ctx.enter_context(tc.tile_pool(name="small", bufs=4))
        cols_p = ctx.enter_context(tc.tile_pool(name="cols", bufs=1))
        stage0_p = ctx.enter_context(tc.tile_pool(name="stage0", bufs=2))
        scans_p = ctx.enter_context(tc.tile_pool(name="scans", bufs=2))
        scr_p = ctx.enter_context(tc.tile_pool(name="scr", bufs=2))
        psa_p = ctx.enter_context(tc.tile_pool(name="psa", bufs=2, space="PSUM"))
        psb_p = ctx.enter_context(tc.tile_pool(name="psb", bufs=2, space="PSUM"))

        # ---------------- constants (ordered so stage0/A inputs land first)
        ident = const_p.tile([128, 128], F32)
        make_identity(nc, ident[:])
        eps_t = const_p.tile([128, 1], F32)
        nc.vector.memset(eps_t[:], EPS_LN)
        ones_t = const_p.tile([128, 1], F32)
        nc.vector.memset(ones_t[:], 1.0)
        # iota8x8 = [0, 8, 16, ..., 56] per partition
        iota8 = const_p.tile([128, 8], F32)
        for h in range(8):
            nc.vector.memset(iota8[:, h:h + 1], float(8 * h))
        csum_b = const_p.tile([128, EDIM], F32)
        _ps_ap = projsum[:]
        nc.sync.dma_start(
            csum_b[:],
            bass.AP(tensor=_ps_ap.tensor, offset=_ps_ap.offset,
                    ap=[[0, 128], _ps_ap.ap[-1]]))

        # projection chunks [128,16] x2 + [64,16]
        pj0 = const_p.tile([128, EDIM], F32)
        pj1 = const_p.tile([128, EDIM], F32)
        pj2 = const_p.tile([64, EDIM], F32)
        nc.sync.dma_start(pj0[:], proj[0:128, :])
        nc.sync.dma_start(pj1[:], proj[128:256, :])
        nc.sync.dma_start(pj2[:], proj[256:320, :])
        pj = [pj0, pj1, pj2]

        st_sb = []
        for kc, (k0, k1) in enumerate([(0, 128), (128, 256), (256, 320)]):
            t_ = const_p.tile([k1 - k0, R], F32, name=f"stackT_{kc}")
            nc.sync.dma_start(t_[:], stackT[k0:k1, :])
            st_sb.append(t_)

        ek_sb = const_p.tile([128, V], BF16)
        nc.sync.dma_start(ek_sb[:], ek[:])

        et_sb = []
        for kc in range(4):
            t_ = const_p.tile([128, R], BF16, name=f"encT_{kc}")
            nc.sync.dma_start(t_[:], encT[kc * 128:(kc + 1) * 128, :])
            et_sb.append(t_)

        # W last, on the gpsimd DGE queue (parallel to the sync queue)
        w_sb = []
        for kc in range(4):
            wt_ = const_p.tile([128, V], BF16, name=f"w_sb_{kc}")
            nc.gpsimd.dma_start(wt_[:], w[kc * 128:(kc + 1) * 128, :])
            w_sb.append(wt_)

        xk_bufs = []
        for i_ in range(RT):
            xkt = const_p.tile([128, 128], BF16, name=f"xk_{i_}")
            nc.vector.memset(xkt[:], 0.0)
            xk_bufs.append(xkt)

        s_cols = cols_p.tile([128, RT, NVC], F32)
        c_cols = cols_p.tile([128, RT, NVC], F32)
        ltgt_cols = cols_p.tile([128, RT], F32)
        bias_cols = cols_p.tile([128, RT], F32)
        mask_cols = cols_p.tile([128, RT], F32)
        st_cols = cols_p.tile([128, RT], F32)
        cnt_cols = cols_p.tile([128, RT], F32)
        red_cols = cols_p.tile([128, 2 * RT], F32)

        # ---------------- stage 0 -----------------------------------------
        # pass 1: stats + ln(var+eps) for all tiles (Ln's batched)
        mv_t = []
        lnv_t = []
        for rt in range(RT):
            stk_t = stage0_p.tile([128, SF], F32, name="stk_t")
            nc.sync.dma_start(stk_t[:], stack_rows[rt * 128:(rt + 1) * 128, :])
            stats = small_p.tile([128, 6], F32, name="stats")
            nc.vector.bn_stats(stats[:], stk_t[:])
            mv = small_p.tile([128, 2], F32, name=f"mv_{rt}", bufs=RT)
            nc.vector.bn_aggr(mv[:], stats[:])
            lnv = small_p.tile([128, 1], F32, name=f"lnv_{rt}", bufs=RT)
            nc.scalar.activation(lnv[:], mv[:, 1:2], AF.Ln, bias=eps_t[:])
            mv_t.append(mv)
            lnv_t.append(lnv)
        # pass 2: rstd = exp(-0.5*lnv); projection + affine LN fix + Kahan
        for rt in range(RT):
            c0 = rt * 128
            rstd = small_p.tile([128, 1], F32, name="rstd")
            nc.scalar.activation(rstd[:], lnv_t[rt][:], AF.Exp, scale=-0.5)

            psq = psa_p.tile([128, EDIM], F32, name="psq", tag="biga")
            for kc, (k0, k1) in enumerate([(0, 128), (128, 256), (256, 320)]):
                nc.tensor.matmul(psq[:], st_sb[kc][:, c0:c0 + 128], pj[kc][:],
                                 start=(kc == 0), stop=(kc == 2))
            mu_c = small_p.tile([128, EDIM], F32, name="mu_c")
            nc.vector.tensor_scalar(mu_c[:], csum_b[:], mv_t[rt][:, 0:1], None,
                                    ALU.mult)
            x_t = small_p.tile([128, EDIM], F32, name="x_t")
            nc.vector.tensor_tensor(out=x_t[:], in0=psq[:], in1=mu_c[:],
                                    op=ALU.subtract)
            nc.vector.tensor_scalar(x_t[:], x_t[:], rstd[:], None, ALU.mult)

            pst = psa_p.tile([16, 128], F32, name="pst", tag="biga")
            nc.tensor.transpose(pst[:], x_t[:], ident[:])

            xk = xk_bufs[rt]
            xh_f = small_p.tile([16, 128], F32, name="xh_f")
            nc.vector.tensor_copy(xk[0:16, :], pst[:])           # hi (cast)
            nc.vector.tensor_copy(xh_f[:], xk[0:16, :])          # back to f32
            nc.vector.tensor_tensor(out=xh_f[:], in0=pst[:], in1=xh_f[:],
                                    op=ALU.subtract)             # residual
            nc.vector.tensor_copy(xk[32:48, :], xh_f[:])         # lo (cast)
            nc.vector.tensor_copy(xk[64:80, :], xk[0:16, :])
            nc.vector.tensor_copy(xk[96:112, :], xk[32:48, :])

        # ---------------- pipelined stage A(rt+1) / prep(rt) / B(rt) ----
        codes_i = [None] * RT
        codes_f = [None] * RT

        def emit_stage_a(rt):
            xk = xk_bufs[rt]
            mcat = scans_p.tile([128, 8 * NVC], F32, name="mcat")
            icat = scans_p.tile([128, 8 * NVC], U32, name="icat")
            for h in range(NVC):
                psd = psa_p.tile([128, VC], F32, name="psd", tag="biga")
                for j in range(VC // 512):
                    nc.tensor.matmul(
                        psd[:, j * 512:(j + 1) * 512], xk[:],
                        ek_sb[:, h * VC + j * 512:h * VC + (j + 1) * 512],
                        start=True, stop=True)
                nc.vector.max(mcat[:, 8 * h:8 * h + 8], psd[:])
                nc.vector.max_index(icat[:, 8 * h:8 * h + 8],
                                    mcat[:, 8 * h:8 * h + 8], psd[:])
            # cross-chunk combine: global max position p = 8*h_win in mcat
            gm8 = small_p.tile([128, 8], F32, name="gm8")
            nc.vector.max(gm8[:], mcat[:])
            gp8 = small_p.tile([128, 8], U32, name="gp8")
            nc.vector.max_index(gp8[:], gm8[:], mcat[:])
            pf = small_p.tile([128, 1], F32, name="pf")
            nc.vector.tensor_copy(pf[:], gp8[:, 0:1])
            icf = small_p.tile([128, 8 * NVC], F32, name="icf")
            nc.vector.tensor_copy(icf[:], icat[:])
            # one-hot over chunks: eqv[:, h] = (pf == 8h)
            eqv = small_p.tile([128, NVC], F32, name="eqv")
            nc.vector.tensor_scalar(eqv[:], iota8[:, 0:NVC], pf[:], None,
                                    ALU.is_equal)
            sel = small_p.tile([128, NVC], F32, name="sel")
            nc.vector.tensor_tensor(out=sel[:], in0=eqv[:],
                                    in1=icf[:, 0:8 * NVC:8], op=ALU.mult)
            lsel = small_p.tile([128, 1], F32, name="lsel")
            nc.vector.reduce_sum(lsel[:], sel[:], axis=mybir.AxisListType.X)
            # code = VC*h + l = (VC/8)*p + l
            cf = small_p.tile([128, 1], F32, name=f"cf_{rt}", bufs=RT)
            nc.vector.scalar_tensor_tensor(
                out=cf[:], in0=pf[:], scalar=float(VC // 8), in1=lsel[:],
                op0=ALU.mult, op1=ALU.add)
            ci = small_p.tile([128, 1], U32, name=f"ci_{rt}", bufs=RT)
            nc.vector.tensor_copy(ci[:], cf[:])
            codes_i[rt] = ci
            codes_f[rt] = cf

        def emit_prep(rt):
            nc.sync.dma_start(mask_cols[:, rt:rt + 1],
                              maskce[rt * 128:(rt + 1) * 128, :])
            g_t = scr_p.tile([128, D], F32, name="g_t")
            nc.gpsimd.indirect_dma_start(
                out=g_t[:], out_offset=None, in_=wt[:],
                in_offset=IndirectOffsetOnAxis(ap=codes_i[rt][:, :1], axis=0))
            er_t = scr_p.tile([128, D], F32, name="er_t")
            nc.sync.dma_start(er_t[:], enc_rows[rt * 128:(rt + 1) * 128, :])
            prod = scr_p.tile([128, D], F32, name="prod")
            nc.vector.scalar_tensor_tensor(
                out=prod[:], in0=er_t[:], scalar=1.0, in1=g_t[:],
                op0=ALU.mult, op1=ALU.mult,
                accum_out=ltgt_cols[:, rt:rt + 1])
            # exp2 bias = -SHARP*(ltgt + MARGIN)
            nc.vector.tensor_scalar(bias_cols[:, rt:rt + 1],
                                    ltgt_cols[:, rt:rt + 1],
                                    -SHARP, -SHARP * MARGIN,
                                    ALU.mult, ALU.add)

        def emit_stage_b(rt):
            for h in range(NVC):
                psl = psb_p.tile([128, VC], F32, name="psl", tag="bigb")
                for kc in range(4):
                    for j in range(VC // 512):
                        nc.tensor.matmul(
                            psl[:, j * 512:(j + 1) * 512],
                            et_sb[kc][:, rt * 128:(rt + 1) * 128],
                            w_sb[kc][:, h * VC + j * 512:h * VC + (j + 1) * 512],
                            start=(kc == 0), stop=(kc == 3))
                exp_t = scr_p.tile([128, VC], BF16, name="exp_t")
                nc.scalar.activation(exp_t[:], psl[:], AF.Exp,
                                     accum_out=s_cols[:, rt, h:h + 1])
                shp_t = scr_p.tile([128, VC], BF16, name="shp_t")
                nc.scalar.activation(shp_t[:], psl[:], AF.Exp,
                                     scale=SHARP,
                                     bias=bias_cols[:, rt:rt + 1],
                                     accum_out=c_cols[:, rt, h:h + 1])

        emit_stage_a(0)
        emit_stage_a(1)
        emit_prep(0)
        for rt in range(RT):
            emit_stage_b(rt)
            if rt + 2 < RT:
                emit_stage_a(rt + 2)
            if rt + 1 < RT:
                emit_prep(rt + 1)
            # per-tile partial reductions (DVE, schedule-anywhere)
            nc.vector.reduce_sum(st_cols[:, rt:rt + 1], s_cols[:, rt, :],
                                 axis=mybir.AxisListType.X)
            nc.vector.reduce_sum(cnt_cols[:, rt:rt + 1], c_cols[:, rt, :],
                                 axis=mybir.AxisListType.X)

        # ---------------- batched finalize -------------------------------
        lnS = small_p.tile([128, RT], F32, name="lnS")
        nc.scalar.activation(lnS[:], st_cols[:], AF.Ln)
        nll = small_p.tile([128, RT], F32, name="nll")
        nc.vector.tensor_tensor(out=nll[:], in0=lnS[:], in1=ltgt_cols[:],
                                op=ALU.subtract)
        nc.vector.tensor_tensor(out=red_cols[:, 0:RT], in0=nll[:],
                                in1=mask_cols[:], op=ALU.mult)
        corr = small_p.tile([128, RT], F32, name="corr")
        nc.vector.tensor_scalar(corr[:], cnt_cols[:], 0.5, None, ALU.is_lt)
        nc.vector.tensor_tensor(out=red_cols[:, RT:2 * RT], in0=corr[:],
                                in1=mask_cols[:], op=ALU.mult)

        for rt in range(RT):
            pidx_f = small_p.tile([128, 1], F32, name="pidx_f")
            nc.vector.tensor_tensor(out=pidx_f[:], in0=codes_f[rt][:],
                                    in1=mask_cols[:, rt:rt + 1], op=ALU.mult)
            pidx = small_p.tile([128, 1], I32, name="pidx")
            nc.vector.tensor_copy(pidx[:], pidx_f[:])
            nc.gpsimd.indirect_dma_start(
                out=out_pres[:], out_offset=IndirectOffsetOnAxis(
                    ap=pidx[:, :1], axis=0),
                in_=ones_t[:, :], in_offset=None)

        # ---------------- partition reduction ---------------------------
        psr = psa_p.tile([1, 2 * RT], F32, name="psr", tag="biga")
        nc.tensor.matmul(psr[:], ones_t[:], red_cols[:], start=True, stop=True)
        fin = small_p.tile([1, 8], F32, name="fin")
        nc.vector.reduce_sum(fin[:, 0:1], psr[0:1, 0:RT], axis=mybir.AxisListType.X)
        nc.vector.reduce_sum(fin[:, 1:2], psr[0:1, RT:2 * RT],
                             axis=mybir.AxisListType.X)
        nc.vector.memset(fin[:, 2:8], 0.0)
        nc.sync.dma_start(out_stats[:], fin[:])

    nc.compile()
    _NC_CACHE["nc"] = nc
    return nc


def _row_map(lengths):
    """Valid CE rows (b, j): enc frame j, target frame j+1; j+1 <= L_b-1."""
    L = np.asarray(lengths).astype(np.int64) // STRIDE
    bs, js = [], []
    for b in range(B):
        n = int(L[b]) - 1
        bs.extend([b] * n)
        js.extend(range(n))
    nvalid = len(bs)
    pad = NCORES * R - nvalid
    assert pad >= 0, f"too many valid rows: {nvalid}"
    bs = np.array(bs + [0] * pad, dtype=np.int64)
    js = np.array(js + [0] * pad, dtype=np.int64)
    vm = np.zeros(NCORES * R, dtype=bool)
    vm[:nvalid] = True
    return bs, js, vm, nvalid


def _prep_core_inputs(inputs, core, row_map):
    feats = np.asarray(inputs["feats"])
    enc = np.asarray(inputs["encoder_out"])
    bs, js, vm, _ = row_map
    sl = slice(core * R, (core + 1) * R)
    b_c, j_c, v_c = bs[sl], js[sl], vm[sl]

    fb = feats.reshape(B, N, SF)
    stack_rows = fb[b_c, j_c + 1].astype(np.float32)
    stack_rows[~v_c] = 0.0
    enc_r = enc[b_c, j_c].astype(np.float32)
    enc_r[~v_c] = 0.0
    maskce = v_c.astype(np.float32).reshape(R, 1)

    return {
        "stack_rows": np.ascontiguousarray(stack_rows),
        "stackT": np.ascontiguousarray(stack_rows.T),
        "encT": np.ascontiguousarray(enc_r.T.astype(ml_dtypes.bfloat16)),
        "enc_rows": np.ascontiguousarray(enc_r),
        "maskce": np.ascontiguousarray(maskce),
    }


def _prep_shared_inputs(inputs):
    proj = np.asarray(inputs["projection"], dtype=np.float32)
    emb = np.asarray(inputs["embeddings"], dtype=np.float32)
    top = np.asarray(inputs["top_n_out"], dtype=np.float32)

    projsum = proj.sum(0, keepdims=True).astype(np.float32)  # [1, 16]

    Et = np.ascontiguousarray(emb[:, 0, :].T, dtype=np.float32)  # [16, V]
    Eh = Et.astype(ml_dtypes.bfloat16).astype(np.float32)
    El = (Et - Eh).astype(ml_dtypes.bfloat16).astype(np.float32)
    Z = np.zeros_like(Eh)
    # row pairing with x tile [xh;0;xl;0;xh;0;xl;0]: hh + lh + hl + ll
    Ek = np.concatenate(
        [Eh, Z, Eh, Z, El, Z, El, Z], axis=0).astype(ml_dtypes.bfloat16)

    W = np.ascontiguousarray(top[0, 0], dtype=np.float32)        # [D, V]
    Wt = np.ascontiguousarray(W.T)                               # [V, D]
    return {
        "proj": np.ascontiguousarray(proj),
        "projsum": projsum,
        "Ek": np.ascontiguousarray(Ek),
        "W": np.ascontiguousarray(W.astype(ml_dtypes.bfloat16)),
        "Wt": Wt,
    }


def _combine(results, inputs, row_map):
    _, _, _, nvalid = row_map
    num_codes = float(nvalid)

    nll_sum = 0.0
    corr_sum = 0.0
    pres = np.zeros(V, dtype=bool)
    for r in results:
        st = np.asarray(r["out_stats"]).reshape(-1)
        nll_sum += float(st[0])
        corr_sum += float(st[1])
        pres |= np.asarray(r["out_pres"]).reshape(-1) > 0.0
    # reference scatters index 0 for every masked grid row; those exist
    # whenever num_codes < B*(N-1) (always here)
    if nvalid < B * (N - 1):
        pres[0] = True

    loss = np.float32(nll_sum / num_codes)
    acc = np.float32(corr_sum / num_codes)
    uniq = np.float32(pres.sum())
    return np.array([loss, acc, np.float32(num_codes), uniq], dtype=np.float32)


def _run(inputs, trace=False):
    from concourse.bass_utils import run_bass_kernel_spmd
    nc = _build_program()
    row_map = _row_map(inputs["feats_lengths"])
    shared = _prep_shared_inputs(inputs)
    in_maps = []
    for core in range(NCORES):
        m = dict(shared)
        m.update(_prep_core_inputs(inputs, core, row_map))
        in_maps.append(m)
    res = run_bass_kernel_spmd(nc, in_maps, core_ids=list(range(NCORES)),
                               trace=trace)
    out = _combine(res.results, inputs, row_map)
    return out, res


def _run_sim(inputs, core=0):
    """Single-core simulator run (correctness debugging)."""
    from concourse.bass_interp import CoreSim
    nc = _build_program()
    row_map = _row_map(inputs["feats_lengths"])
    m = dict(_prep_shared_inputs(inputs))
    m.update(_prep_core_inputs(inputs, core, row_map))
    sim = CoreSim(nc, require_finite=False, require_nnan=False)
    for k, v in m.items():
        sim.tensor(k)[:] = v
    sim.simulate()
    return {k: np.array(sim.tensor(k)) for k in ("out_stats", "out_pres")}


def kernel(**inputs) -> np.ndarray:
    out, _ = _run(inputs, trace=False)
    return out
